# revision 4
# baseline (speedup 1.0000x reference)
"""CRF negative-log-likelihood loss kernel for Trainium2 (8 NeuronCores), v2.

Data-parallel over batch (64 seqs -> 8 cores x 8 seqs). The forward
(log-partition) scan runs in the exp domain as a FUSED forward+backward
meet-in-the-middle chain, halving sequential depth 511 -> 255:

    state u_k = [alpha_k ; gamma_k]   (128 partitions x 8 seqs, bf16)
    alpha_k = e_k (.) (T^T alpha_{k-1})          (fwd, rows 0:64)
    gamma_k = e_{511-k} (.) (T gamma_{k-1})      (bwd, rows 64:128)
    Z_b = sum_i alpha_255[i,b] * (T gamma_255)[i,b]

Partition layout per half (32-aligned access rule): tags 0-31 at rows
0-31, the half's column-sum row at 32 (fed by a ones column in the
stationary -> per-step sums for free), tags 32-49 at rows 34-51, rest
padded with exp(-30000)=0. One [128,128] bf16 stationary (blockdiag
expT / expT^T, host-prepermuted in log space) gives ONE matmul + ONE
DVE multiply per superstep. Emissions are max-shifted on the host
(kappa preprocessing; shifts summed back in via the ksum input); fp32
range kept safe by per-column rescales every R=9 steps, folded into the
emission buffer LAZY=4 steps later (off the critical path). Validated
on the actual input distribution: max |ln m| ~ 31 < 44 (Ln range).

Gold score on-device: one-hot/emission work on GpSimd (idle during the
scan), (prev,tag) count matmuls on PE after the scan.
"""

import numpy as np

TAG = 50
START = TAG - 2
STOP = TAG - 1
B, S = 64, 512
NCORES = 8
BPC = B // NCORES  # sequences per core
HALF = S // 2      # supersteps
NCH = 4
CH = S // NCH
R = 9              # rescale period (host kappa shift: max |ln m| = 31.3 < 44)
LAZY = 4           # rescale factor applied LAZY steps later
NEG = -30000.0     # exp(NEG) == 0 padding

# within-half row of tag t (sum row sits at 32)
_RMAP = np.array([t if t < 32 else t + 2 for t in range(TAG)])

_COMPILED = {}
LAST_RESULTS = None
LAST_IN_MAPS = None


def _host_consts(transitions):
    """Host-prepermuted log-space stationary + init column + indicators."""
    T = transitions.astype(np.float32)
    wlog = np.full((128, 128), NEG, dtype=np.float32)
    r = _RMAP
    # fwd block: out row r(j) = sum_i T[i,j] v[r(i)]
    wlog[np.ix_(r, r)] = T
    # bwd block: out row 64+r(i) = sum_j T[i,j] v[64+r(j)]
    wlog[np.ix_(64 + r, 64 + r)] = T.T
    # per-half sum columns (exp(0)=1 weights)
    wlog[r, 32] = 0.0
    wlog[64 + r, 96] = 0.0
    initlog = np.full((128, 1), NEG, dtype=np.float32)
    initlog[r, 0] = T[START, :]
    initlog[64 + r, 0] = T[:, STOP]
    indf = np.zeros((1, 128), dtype=np.float32)
    indf[0, r] = 1.0
    indb = np.zeros((1, 128), dtype=np.float32)
    indb[0, 64 + r] = 1.0
    return wlog, initlog, indf, indb


def _build(reps=1, gp="big", wdt="bf16", no_gold=False, no_resc=False,
           ps_bufs=4, v_bufs=3, lazy=LAZY, rr=R, resc_eng="dve"):
    import concourse.bass as bass
    import concourse.bacc as bacc
    import concourse.tile as tile
    from concourse import mybir

    f32 = mybir.dt.float32
    bf16 = mybir.dt.bfloat16
    i32 = mybir.dt.int32
    AF = mybir.ActivationFunctionType
    ALU = mybir.AluOpType
    AX = mybir.AxisListType

    nc = bacc.Bacc("TRN2", target_bir_lowering=False, debug=False,
                   enable_asserts=False, num_devices=NCORES)

    # efraw: fused-layout raw feats [128, 256*8], col = k*8+b:
    #   row rmap(g)    = feats[b, k, g]      (fwd)
    #   row 64+rmap(g) = feats[b, 511-k, g]  (bwd), pad rows = NEG
    efraw_d = nc.dram_tensor("efraw", [128, HALF * BPC], f32,
                             kind="ExternalInput")
    wlog_d = nc.dram_tensor("wlog", [128, 128], f32, kind="ExternalInput")
    initlog_d = nc.dram_tensor("initlog", [128, 1], f32,
                               kind="ExternalInput")
    indf_d = nc.dram_tensor("indf", [1, 128], f32, kind="ExternalInput")
    indb_d = nc.dram_tensor("indb", [1, 128], f32, kind="ExternalInput")
    feats = nc.dram_tensor("feats", [BPC, S, TAG], f32, kind="ExternalInput")
    tagsf = nc.dram_tensor("tagsf", [BPC, S], f32, kind="ExternalInput")
    prevf = nc.dram_tensor("prevf", [BPC, S], f32, kind="ExternalInput")
    endf = nc.dram_tensor("endf", [BPC, 1], f32, kind="ExternalInput")
    ksum_d = nc.dram_tensor("ksum", [1, BPC], f32, kind="ExternalInput")
    trans = nc.dram_tensor("trans", [TAG, TAG], f32, kind="ExternalInput")
    out = nc.dram_tensor("out", [1, 16], f32, kind="ExternalOutput")

    with tile.TileContext(nc) as tc:
        with tc.tile_pool(name="const", bufs=1) as cpool, \
             tc.tile_pool(name="big", bufs=1) as bigpool, \
             tc.tile_pool(name="oh", bufs=6) as ohpool, \
             tc.tile_pool(name="small", bufs=4) as spool, \
             tc.tile_pool(name="v", bufs=v_bufs) as vpool, \
             tc.tile_pool(name="ps_s", bufs=ps_bufs, space="PSUM") as ps_s, \
             tc.tile_pool(name="ps_m", bufs=2, space="PSUM") as ps_m, \
             tc.tile_pool(name="ps_cnt", bufs=1, space="PSUM") as ps_cnt, \
             tc.tile_pool(name="ps_z", bufs=1, space="PSUM") as ps_z:

            # ---------- constants ----------
            iota_row_i = cpool.tile([128, 128], i32)
            nc.gpsimd.iota(iota_row_i[:], pattern=[[1, 128]], base=0,
                           channel_multiplier=0)
            iota_row_f = cpool.tile([128, 128], f32)
            nc.vector.tensor_copy(iota_row_f[:], iota_row_i[:])
            ones64 = cpool.tile([64, 1], f32)
            nc.vector.memset(ones64[:], 1.0)
            ones50 = cpool.tile([TAG, 1], f32)
            nc.vector.memset(ones50[:], 1.0)
            ones128 = cpool.tile([128, 1], f32)
            nc.vector.memset(ones128[:], 1.0)
            oh_stop = cpool.tile([BPC, TAG], f32)
            nc.vector.tensor_scalar(oh_stop[:], iota_row_f[:BPC, :TAG],
                                    float(STOP), None, op0=ALU.is_equal)
            iotarep_i = cpool.tile([128, NCH * BPC * TAG], i32)
            nc.gpsimd.iota(iotarep_i[:], pattern=[[0, NCH * BPC], [1, TAG]],
                           base=0, channel_multiplier=0)
            iotarep = cpool.tile([128, NCH * BPC * TAG], f32)
            nc.vector.tensor_copy(iotarep[:], iotarep_i[:])

            osb_prev = None
            for _rep in range(reps):
                # ---------- input DMAs ----------
                HB = HALF * BPC // 2
                ef0 = bigpool.tile([128, HB], f32, tag="ef0", name="ef0")
                nc.sync.dma_start(ef0[:], efraw_d[:, 0:HB])
                ef1 = bigpool.tile([128, HB], f32, tag="ef1", name="ef1")
                nc.sync.dma_start(ef1[:], efraw_d[:, HB:])
                wl = cpool.tile([128, 128], f32, tag=f"wl{_rep}")
                nc.sync.dma_start(wl[:], wlog_d[:, :])
                il = cpool.tile([128, 1], f32, tag=f"il{_rep}")
                nc.sync.dma_start(il[:], initlog_d[:, :])
                IndF = cpool.tile([1, 128], f32, tag=f"if{_rep}")
                nc.sync.dma_start(IndF[:], indf_d[:, :])
                IndB = cpool.tile([1, 128], f32, tag=f"ib{_rep}")
                nc.sync.dma_start(IndB[:], indb_d[:, :])
                tsb = cpool.tile([TAG, TAG], f32, tag=f"ts{_rep}")
                nc.sync.dma_start(tsb[:], trans[:, :])
                tag_all = cpool.tile([128, NCH * BPC], f32, tag=f"tg{_rep}")
                prev_all = cpool.tile([128, NCH * BPC], f32, tag=f"pv{_rep}")
                for c in range(NCH):
                    nc.sync.dma_start(
                        tag_all[:, c * BPC:(c + 1) * BPC],
                        tagsf[:, bass.ts(c, CH)].rearrange("b t -> t b"))
                    nc.sync.dma_start(
                        prev_all[:, c * BPC:(c + 1) * BPC],
                        prevf[:, bass.ts(c, CH)].rearrange("b t -> t b"))
                endsb = cpool.tile([BPC, 1], f32, tag=f"en{_rep}")
                nc.sync.dma_start(endsb[:], endf[:, :])
                ksb = cpool.tile([1, BPC], f32, tag=f"ks{_rep}")
                nc.sync.dma_start(ksb[:], ksum_d[:, :])
                fbuf = bigpool.tile([128, NCH * BPC * TAG], f32,
                                    tag="fb", name="fb")
                fb3 = fbuf[:].rearrange("p (c b g) -> p c b g", c=NCH, b=BPC)
                for c in range(NCH):
                    nc.sync.dma_start(
                        fb3[:, c, :, :],
                        feats[:, bass.ts(c, CH), :].rearrange(
                            "b t g -> t b g"))

                # ---------- stationary + init (exp of host log consts) -----
                vdt = bf16 if wdt == "bf16" else f32
                Wfb = cpool.tile([128, 128], vdt, tag=f"W{_rep}")
                nc.scalar.activation(Wfb[:], wl[:], AF.Exp)
                initcol = cpool.tile([128, 1], f32, tag=f"ic{_rep}")
                nc.scalar.activation(initcol[:], il[:], AF.Exp)

                # ---------- exp(feats) in place, first-needed half first ----
                nc.scalar.activation(ef0[:], ef0[:], AF.Exp)
                nc.scalar.activation(ef1[:], ef1[:], AF.Exp)
                ef0_v = ef0[:].rearrange("p (t b) -> p t b", b=BPC)
                ef1_v = ef1[:].rearrange("p (t b) -> p t b", b=BPC)

                def ef_at(k):
                    if k < HALF // 2:
                        return ef0_v[:, k, :]
                    return ef1_v[:, k - HALF // 2, :]

                # ---------- gold one-hots + emission terms ----------
                # gp=True: on GpSimd, issued before the scan (its queue is
                # otherwise idle). gp=False: on DVE, issued after the scan
                # TTs so they do not delay the chain.
                if gp == "gp":
                    emitbuf = cpool.tile([1, NCH * BPC], f32,
                                         tag=f"em{_rep}")
                else:
                    emitbuf = cpool.tile([128, NCH * BPC], f32,
                                         tag=f"em{_rep}")
                oTs = {}
                oPs = {}
                oh_end = cpool.tile([BPC, TAG], f32, tag=f"oe{_rep}")

                oT_all = None

                def do_onehots_big():
                    nonlocal oT_all
                    NC_ = NCH * BPC
                    oT_all = bigpool.tile([128, NC_ * TAG], f32,
                                          tag="oTa",
                                          name="oTa")
                    oP_all = bigpool.tile([128, NC_ * TAG], f32,
                                          tag="oPa",
                                          name="oPa")
                    tag_b = tag_all[:].rearrange(
                        "p (a o) -> p a o", o=1).broadcast_to([128, NC_, TAG])
                    prev_b = prev_all[:].rearrange(
                        "p (a o) -> p a o", o=1).broadcast_to([128, NC_, TAG])
                    i3 = iotarep[:].rearrange("p (a g) -> p a g", g=TAG)
                    nc.vector.tensor_tensor(
                        oT_all[:].rearrange("p (a g) -> p a g", g=TAG),
                        i3, tag_b, op=ALU.is_equal)
                    nc.vector.tensor_tensor(
                        oP_all[:].rearrange("p (a g) -> p a g", g=TAG),
                        i3, prev_b, op=ALU.is_equal)
                    em_all = bigpool.tile([128, NC_ * TAG], f32,
                                          tag="ema",
                                          name="ema")
                    nc.vector.tensor_tensor(em_all[:], fbuf[:], oT_all[:],
                                            op=ALU.mult)
                    nc.vector.tensor_reduce(
                        emitbuf[:],
                        em_all[:].rearrange("p (a g) -> p a g", g=TAG),
                        axis=AX.X, op=ALU.add)
                    for col in range(NC_):
                        oTs[col] = oT_all[:, col * TAG:(col + 1) * TAG]
                        oPs[col] = oP_all[:, col * TAG:(col + 1) * TAG]
                    nc.vector.tensor_scalar(oh_end[:],
                                            iota_row_f[:BPC, :TAG],
                                            endsb[:], None,
                                            op0=ALU.is_equal)

                def do_onehots():
                    if gp == "big":
                        do_onehots_big()
                        return
                    eng = nc.gpsimd if gp == "gp" else nc.vector
                    for c in range(NCH):
                        for b in range(BPC):
                            col = c * BPC + b
                            oT = ohpool.tile([128, TAG], f32, tag="oT",
                                             name=f"oT{_rep}_{col}")
                            eng.tensor_scalar(
                                oT[:], iota_row_f[:, :TAG],
                                tag_all[:, col:col + 1], None,
                                op0=ALU.is_equal)
                            oP = ohpool.tile([128, TAG], f32, tag="oP",
                                             name=f"oP{_rep}_{col}")
                            eng.tensor_scalar(
                                oP[:], iota_row_f[:, :TAG],
                                prev_all[:, col:col + 1], None,
                                op0=ALU.is_equal)
                            em = ohpool.tile([128, TAG], f32, tag="em")
                            eng.tensor_tensor(em[:], fb3[:, c, b, :],
                                              oT[:], op=ALU.mult)
                            if gp == "gp":
                                eng.tensor_reduce(emitbuf[:, col:col + 1],
                                                  em[:], axis=AX.XYZWC,
                                                  op=ALU.add)
                            else:
                                eng.tensor_reduce(emitbuf[:, col:col + 1],
                                                  em[:], axis=AX.X,
                                                  op=ALU.add)
                            oTs[col] = oT
                            oPs[col] = oP
                    eng.tensor_scalar(oh_end[:], iota_row_f[:BPC, :TAG],
                                      endsb[:], None, op0=ALU.is_equal)

                if gp == "gp" and not no_gold:
                    do_onehots()

                # ---------- rescale bookkeeping ----------
                FINAL_LAZY = 2
                final_k = HALF - 1 - FINAL_LAZY
                resc = {}
                for k in range(1, HALF):
                    if (k % rr == rr - 1 and k + lazy <= HALF - 1
                            and k + lazy != final_k + FINAL_LAZY
                            and k != final_k):
                        resc[k] = lazy
                resc[final_k] = FINAL_LAZY
                resc_steps = sorted(resc)
                if no_resc:
                    resc = {}
                    resc_steps = []
                nresc = len(resc_steps)
                lnbuf0 = cpool.tile([1, max(nresc, 1) * BPC], f32,
                                    tag=f"lb0{_rep}")
                lnbuf1 = cpool.tile([1, max(nresc, 1) * BPC], f32,
                                    tag=f"lb1{_rep}")
                if nresc == 0:
                    nc.vector.memset(lnbuf0[:], 0.0)
                    nc.vector.memset(lnbuf1[:], 0.0)

                if gp == "big" and not no_gold:
                    do_onehots()

                # ---------- fused forward+backward scan ----------
                v = vpool.tile([128, BPC], vdt, tag="v")
                nc.vector.tensor_scalar(v[:], ef_at(0), initcol[:],
                                        None, op0=ALU.mult)
                folds = {}
                pending = None
                for k in range(1, HALF):
                    s_ps = ps_s.tile([128, BPC], f32, tag="s")
                    nc.tensor.matmul(s_ps[:], Wfb[:], v[:], start=True,
                                     stop=True)
                    src_ap = folds.pop(k, None)
                    if src_ap is None:
                        src_ap = ef_at(k)
                    else:
                        src_ap = src_ap[:]
                    v2 = vpool.tile([128, BPC], vdt, tag="v")
                    nc.vector.tensor_tensor(v2[:], src_ap, s_ps[:],
                                            op=ALU.mult)
                    v = v2
                    if pending is not None:
                        # broadcast matmuls issued AFTER this step's chain
                        # matmul so PE never stalls waiting on rm
                        rm0, rm1, tgt = pending
                        rb_ps = ps_m.tile([128, BPC], f32, tag="m")
                        nc.tensor.matmul(rb_ps[:], IndF[:], rm0[:],
                                         start=True, stop=False,
                                         skip_group_check=True)
                        nc.tensor.matmul(rb_ps[:], IndB[:], rm1[:],
                                         start=False, stop=True,
                                         skip_group_check=True)
                        emod = spool.tile([128, BPC], f32, tag="emod")
                        if resc_eng == "pool":
                            rbs = spool.tile([128, BPC], f32, tag="rbs")
                            nc.scalar.copy(rbs[:], rb_ps[:])
                            nc.gpsimd.tensor_tensor(emod[:], ef_at(tgt),
                                                    rbs[:], op=ALU.mult)
                        else:
                            nc.vector.tensor_tensor(emod[:], ef_at(tgt),
                                                    rb_ps[:], op=ALU.mult)
                        folds[tgt] = emod
                        pending = None
                    if k in resc:
                        # per-half column sums of v_{k-1} from rows 32 / 96
                        ri = resc_steps.index(k)
                        rm0 = spool.tile([1, BPC], f32, tag="rm0")
                        rm1 = spool.tile([1, BPC], f32, tag="rm1")
                        nc.vector.reciprocal(rm0[:], s_ps[32:33, :])
                        nc.vector.reciprocal(rm1[:], s_ps[96:97, :])
                        nc.scalar.activation(
                            lnbuf0[:, ri * BPC:(ri + 1) * BPC],
                            s_ps[32:33, :], AF.Ln)
                        nc.scalar.activation(
                            lnbuf1[:, ri * BPC:(ri + 1) * BPC],
                            s_ps[96:97, :], AF.Ln)
                        pending = (rm0, rm1, k + resc[k])

                # ---------- terminal combine ----------
                s_ps = ps_s.tile([128, BPC], f32, tag="s")
                nc.tensor.matmul(s_ps[:], Wfb[:], v[:], start=True, stop=True)
                zt = spool.tile([64, BPC], f32, tag="zt")
                nc.vector.tensor_tensor(zt[:], v[0:64, :], s_ps[64:128, :],
                                        op=ALU.mult)
                z_ps = ps_z.tile([1, BPC], f32, tag="z")
                nc.tensor.matmul(z_ps[:], ones64[:], zt[:], start=True,
                                 stop=True)
                lnz = spool.tile([1, BPC], f32, tag="lnz")
                nc.scalar.activation(lnz[:], z_ps[:], AF.Ln)
                Csb0 = spool.tile([1, BPC], f32, tag="cs0")
                nc.vector.tensor_reduce(
                    Csb0[:], lnbuf0[:].rearrange("p (r b) -> p b r", b=BPC),
                    axis=AX.X, op=ALU.add)
                Csb1 = spool.tile([1, BPC], f32, tag="cs1")
                nc.vector.tensor_reduce(
                    Csb1[:], lnbuf1[:].rearrange("p (r b) -> p b r", b=BPC),
                    axis=AX.X, op=ALU.add)
                fwd = cpool.tile([1, BPC], f32, tag=f"fw{_rep}")
                nc.vector.tensor_add(fwd[:], lnz[:], Csb0[:])
                nc.vector.tensor_add(fwd[:], fwd[:], Csb1[:])
                nc.vector.tensor_add(fwd[:], fwd[:], ksb[:])
                if osb_prev is not None:
                    # inert data dependency to serialize reps
                    nc.vector.tensor_scalar(fwd[:, 0:1], osb_prev[:, 0:1],
                                            0.0, fwd[:, 0:1],
                                            op0=ALU.mult, op1=ALU.add)

                # ---------- gold: count matmuls (PE tail) ----------
                if gp == "dve" and not no_gold:
                    do_onehots()
                count_ps = ps_cnt.tile([TAG, TAG], f32)
                if no_gold:
                    nc.vector.memset(count_ps[:], 0.0)
                first = True
                if not no_gold:
                    for c in range(NCH):
                        for b in range(BPC):
                            col = c * BPC + b
                            oPa = oPs[col]
                            oTa = oTs[col]
                            if hasattr(oPa, 'tile'):
                                pass
                            try:
                                oPa = oPa[:]
                                oTa = oTa[:]
                            except Exception:
                                pass
                            nc.tensor.matmul(count_ps[:], oPa, oTa,
                                             start=first, stop=False,
                                             skip_group_check=True)
                            first = False
                    nc.tensor.matmul(count_ps[:], oh_end[:], oh_stop[:],
                                     start=False, stop=True,
                                     skip_group_check=True)
                tmul = spool.tile([TAG, TAG], f32, tag="tmul")
                nc.vector.tensor_tensor(tmul[:], tsb[:], count_ps[:],
                                        op=ALU.mult)
                tred = spool.tile([TAG, 1], f32, tag="tred")
                nc.vector.tensor_reduce(tred[:], tmul[:], axis=AX.X,
                                        op=ALU.add)
                gt_ps = ps_z.tile([1, 1], f32, tag="z")
                nc.tensor.matmul(gt_ps[:], ones50[:], tred[:], start=True,
                                 stop=True)
                gemit = spool.tile([1, 1], f32, tag="gem")
                if no_gold:
                    nc.vector.memset(gemit[:], 0.0)
                elif gp == "gp":
                    nc.vector.tensor_reduce(gemit[:], emitbuf[:], axis=AX.X,
                                            op=ALU.add)
                else:
                    ep_ps = ps_z.tile([1, NCH * BPC], f32, tag="z")
                    nc.tensor.matmul(ep_ps[:], ones128[:], emitbuf[:],
                                     start=True, stop=True)
                    nc.vector.tensor_reduce(gemit[:], ep_ps[:], axis=AX.X,
                                            op=ALU.add)

                # ---------- assemble output ----------
                osb = cpool.tile([1, 16], f32, tag=f"osb{_rep}",
                                 name=f"osb{_rep}")
                nc.vector.memset(osb[:], 0.0)
                nc.vector.tensor_copy(osb[:, 0:BPC], fwd[:])
                nc.vector.tensor_copy(osb[:, 8:9], gemit[:])
                nc.vector.tensor_copy(osb[:, 9:10], gt_ps[:])
                nc.sync.dma_start(out[:, :], osb[:])
                osb_prev = osb

    nc.compile()
    return nc, "out"


def _numpy_reference(feats, mask, tags, transitions):
    maskf = mask.astype(np.float64)
    f = feats.astype(np.float64)
    T = transitions.astype(np.float64)
    b, s, t = f.shape
    part = f[:, 0, :] + T[START][None, :]
    for ti in range(1, s):
        cur = part[:, :, None] + T[None, :, :] + f[:, ti, None, :]
        m = cur.max(axis=1)
        cur = m + np.log(np.exp(cur - m[:, None, :]).sum(axis=1))
        part = np.where(mask[:, ti][:, None].astype(bool), cur, part)
    term = part[:, :, None] + T[None, :, :]
    m = term.max(axis=1)
    term = m + np.log(np.exp(term - m[:, None, :]).sum(axis=1))
    forward = term[:, STOP].sum()
    prev = np.concatenate([np.full((b, 1), START, dtype=tags.dtype),
                           tags[:, :-1]], axis=1)
    emit = np.take_along_axis(f, tags[..., None], axis=2)[..., 0]
    tr = T[prev, tags]
    tg = ((emit + tr) * maskf).sum()
    lengths = mask.astype(np.int64).sum(axis=1)
    end_ids = np.take_along_axis(tags, (lengths - 1)[:, None], axis=1)[:, 0]
    gold = tg + T[end_ids, STOP].sum()
    return np.array(forward - gold, dtype=np.float32)


def kernel(feats, mask, tags, transitions):
    global _COMPILED, LAST_RESULTS, LAST_IN_MAPS
    feats = np.asarray(feats, dtype=np.float32)
    mask = np.asarray(mask)
    tags = np.asarray(tags)
    transitions = np.asarray(transitions, dtype=np.float32)

    if not np.all(mask == 1):
        return _numpy_reference(feats, np.asarray(mask, dtype=np.int64),
                                np.asarray(tags, dtype=np.int64), transitions)

    if 1 not in _COMPILED:
        _COMPILED[1] = _build(reps=1)
    nc, out_name = _COMPILED[1]

    tags_i = tags.astype(np.int64)
    prev = np.concatenate(
        [np.full((B, 1), START, dtype=np.int64), tags_i[:, :-1]], axis=1)
    lengths = mask.astype(np.int64).sum(axis=1)
    end_ids = np.take_along_axis(tags_i, (lengths - 1)[:, None], axis=1)[:, 0]

    tagsf = tags_i.astype(np.float32)
    prevf = prev.astype(np.float32)
    endf = end_ids.astype(np.float32).reshape(B, 1)
    wlog, initlog, indf, indb = _host_consts(transitions)

    in_maps = []
    for c in range(NCORES):
        sl = slice(c * BPC, (c + 1) * BPC)
        fshard = feats[sl]                       # (8, 512, 50)
        kap = fshard.max(axis=2)                 # (8, 512) host kappa shift
        fsh = fshard - kap[:, :, None]           # emissions <= 1 in exp dom
        ksum = kap.sum(axis=1).astype(np.float32).reshape(1, BPC)
        fr = fsh.transpose(2, 1, 0)              # (50, 512, 8)
        efraw = np.full((128, HALF, BPC), NEG, dtype=np.float32)
        efraw[_RMAP] = fr[:, :HALF, :]           # fwd: t = k
        efraw[64 + _RMAP] = fr[:, ::-1, :][:, :HALF, :]  # bwd: t = 511-k
        in_maps.append({
            "ksum": ksum,
            "efraw": np.ascontiguousarray(efraw.reshape(128, HALF * BPC)),
            "wlog": wlog,
            "initlog": initlog,
            "indf": indf,
            "indb": indb,
            "feats": np.ascontiguousarray(fshard),
            "tagsf": np.ascontiguousarray(tagsf[sl]),
            "prevf": np.ascontiguousarray(prevf[sl]),
            "endf": np.ascontiguousarray(endf[sl]),
            "trans": transitions,
        })

    from concourse import bass_utils
    res = bass_utils.run_bass_kernel_spmd(nc, in_maps,
                                          core_ids=list(range(NCORES)))
    LAST_RESULTS = res
    LAST_IN_MAPS = in_maps

    total = 0.0
    for c in range(NCORES):
        o = res.results[c][out_name].astype(np.float64)[0]
        total += o[0:BPC].sum() - o[8] - o[9]
    return np.array(total, dtype=np.float32)


# revision 6
# speedup vs baseline: 1.2015x; 1.2015x over previous
"""CRF negative-log-likelihood loss kernel for Trainium2 (8 NeuronCores), v2.

Data-parallel over batch (64 seqs -> 8 cores x 8 seqs). The forward
(log-partition) scan runs in the exp domain as a FUSED forward+backward
meet-in-the-middle chain, halving sequential depth 511 -> 255:

    state u_k = [alpha_k ; gamma_k]   (128 partitions x 8 seqs, bf16)
    alpha_k = e_k (.) (T^T alpha_{k-1})          (fwd, rows 0:64)
    gamma_k = e_{511-k} (.) (T gamma_{k-1})      (bwd, rows 64:128)
    Z_b = sum_i alpha_255[i,b] * (T gamma_255)[i,b]

Partition layout per half (32-aligned access rule): tags 0-31 at rows
0-31, the half's column-sum row at 32 (fed by a ones column in the
stationary -> per-step sums for free), tags 32-49 at rows 34-51, rest
padded with exp(-30000)=0. One [128,128] bf16 stationary (blockdiag
expT / expT^T, host-prepermuted in log space) gives ONE matmul + ONE
DVE multiply per superstep. Emissions are unshifted; fp32 range kept
safe by per-column rescales every R=5 steps, folded into the emission
buffer LAZY=2 steps later (off the critical path). Validated on the
actual input distribution: max |ln m| ~ 26 << 44 (Ln table range).

Gold score on-device: one-hot/emission work on GpSimd (idle during the
scan), (prev,tag) count matmuls on PE after the scan.
"""

import numpy as np

TAG = 50
START = TAG - 2
STOP = TAG - 1
B, S = 64, 512
NCORES = 8
BPC = B // NCORES  # sequences per core
HALF = S // 2      # supersteps
NCH = 4
CH = S // NCH
R = 11             # rescale period (host kappa shift: max |ln m| = 34.4 < 44)
LAZY = 4           # rescale factor applied LAZY steps later
NEG = -30000.0     # exp(NEG) == 0 padding

# within-half row of tag t (sum row sits at 32)
_RMAP = np.array([t if t < 32 else t + 2 for t in range(TAG)])

_COMPILED = {}
LAST_RESULTS = None
LAST_IN_MAPS = None


def _host_consts(transitions):
    """Host-prepermuted log-space stationary + init column + indicators."""
    T = transitions.astype(np.float32)
    wlog = np.full((128, 128), NEG, dtype=np.float32)
    r = _RMAP
    # fwd block: out row r(j) = sum_i T[i,j] v[r(i)]
    wlog[np.ix_(r, r)] = T
    # bwd block: out row 64+r(i) = sum_j T[i,j] v[64+r(j)]
    wlog[np.ix_(64 + r, 64 + r)] = T.T
    # per-half sum columns (exp(0)=1 weights)
    wlog[r, 32] = 0.0
    wlog[64 + r, 96] = 0.0
    initlog = np.full((128, 1), NEG, dtype=np.float32)
    initlog[r, 0] = T[START, :]
    initlog[64 + r, 0] = T[:, STOP]
    indf = np.zeros((1, 128), dtype=np.float32)
    indf[0, r] = 1.0
    indb = np.zeros((1, 128), dtype=np.float32)
    indb[0, 64 + r] = 1.0
    return wlog, initlog, indf, indb


def _build(reps=1, gp="big", wdt="bf16", no_gold=False, no_resc=False,
           ps_bufs=4, v_bufs=3, lazy=LAZY, rr=R, resc_eng="dve"):
    import concourse.bass as bass
    import concourse.bacc as bacc
    import concourse.tile as tile
    from concourse import mybir

    f32 = mybir.dt.float32
    bf16 = mybir.dt.bfloat16
    i32 = mybir.dt.int32
    AF = mybir.ActivationFunctionType
    ALU = mybir.AluOpType
    AX = mybir.AxisListType

    nc = bacc.Bacc("TRN2", target_bir_lowering=False, debug=False,
                   enable_asserts=False, num_devices=NCORES)

    # efraw: fused-layout raw feats [128, 256*8], col = k*8+b:
    #   row rmap(g)    = feats[b, k, g]      (fwd)
    #   row 64+rmap(g) = feats[b, 511-k, g]  (bwd), pad rows = NEG
    efraw_d = nc.dram_tensor("efraw", [128, HALF * BPC], f32,
                             kind="ExternalInput")
    wlog_d = nc.dram_tensor("wlog", [128, 128], f32, kind="ExternalInput")
    initlog_d = nc.dram_tensor("initlog", [128, 1], f32,
                               kind="ExternalInput")
    indf_d = nc.dram_tensor("indf", [1, 128], f32, kind="ExternalInput")
    indb_d = nc.dram_tensor("indb", [1, 128], f32, kind="ExternalInput")
    feats = nc.dram_tensor("feats", [BPC, S, TAG], f32, kind="ExternalInput")
    tagsf = nc.dram_tensor("tagsf", [BPC, S], f32, kind="ExternalInput")
    prevf = nc.dram_tensor("prevf", [BPC, S], f32, kind="ExternalInput")
    endf = nc.dram_tensor("endf", [BPC, 1], f32, kind="ExternalInput")
    ksum_d = nc.dram_tensor("ksum", [1, BPC], f32, kind="ExternalInput")
    trans = nc.dram_tensor("trans", [TAG, TAG], f32, kind="ExternalInput")
    out = nc.dram_tensor("out", [1, 16], f32, kind="ExternalOutput")

    with tile.TileContext(nc) as tc:
        with tc.tile_pool(name="const", bufs=1) as cpool, \
             tc.tile_pool(name="big", bufs=1) as bigpool, \
             tc.tile_pool(name="oh", bufs=6) as ohpool, \
             tc.tile_pool(name="small", bufs=4) as spool, \
             tc.tile_pool(name="v", bufs=v_bufs) as vpool, \
             tc.tile_pool(name="ps_s", bufs=ps_bufs, space="PSUM") as ps_s, \
             tc.tile_pool(name="ps_m", bufs=2, space="PSUM") as ps_m, \
             tc.tile_pool(name="ps_cnt", bufs=1, space="PSUM") as ps_cnt, \
             tc.tile_pool(name="ps_z", bufs=1, space="PSUM") as ps_z:

            # ---------- constants ----------
            iota_row_i = cpool.tile([128, 128], i32)
            nc.gpsimd.iota(iota_row_i[:], pattern=[[1, 128]], base=0,
                           channel_multiplier=0)
            iota_row_f = cpool.tile([128, 128], f32)
            nc.vector.tensor_copy(iota_row_f[:], iota_row_i[:])
            ones64 = cpool.tile([64, 1], f32)
            nc.vector.memset(ones64[:], 1.0)
            ones50 = cpool.tile([TAG, 1], f32)
            nc.vector.memset(ones50[:], 1.0)
            ones128 = cpool.tile([128, 1], f32)
            nc.vector.memset(ones128[:], 1.0)
            oh_stop = cpool.tile([BPC, TAG], f32)
            nc.vector.tensor_scalar(oh_stop[:], iota_row_f[:BPC, :TAG],
                                    float(STOP), None, op0=ALU.is_equal)
            iotarep_i = cpool.tile([128, NCH * BPC * TAG], i32)
            nc.gpsimd.iota(iotarep_i[:], pattern=[[0, NCH * BPC], [1, TAG]],
                           base=0, channel_multiplier=0)
            iotarep = cpool.tile([128, NCH * BPC * TAG], f32)
            nc.vector.tensor_copy(iotarep[:], iotarep_i[:])

            osb_prev = None
            for _rep in range(reps):
                # ---------- input DMAs ----------
                # tiny chain-critical inputs FIRST (SP queue is in-order):
                # the stationary + init column must not wait behind the
                # 256KB emission blocks.
                wl = cpool.tile([128, 128], f32, tag=f"wl{_rep}")
                nc.sync.dma_start(wl[:], wlog_d[:, :])
                il = cpool.tile([128, 1], f32, tag=f"il{_rep}")
                nc.sync.dma_start(il[:], initlog_d[:, :])
                NEB = 4
                EB = HALF * BPC // NEB
                efts = []
                for i in range(NEB):
                    eft = bigpool.tile([128, EB], f32, tag=f"eq{i}",
                                       name=f"eq{i}")
                    nc.sync.dma_start(eft[:],
                                      efraw_d[:, i * EB:(i + 1) * EB])
                    efts.append(eft)
                IndF = cpool.tile([1, 128], f32, tag=f"if{_rep}")
                nc.sync.dma_start(IndF[:], indf_d[:, :])
                IndB = cpool.tile([1, 128], f32, tag=f"ib{_rep}")
                nc.sync.dma_start(IndB[:], indb_d[:, :])
                tsb = cpool.tile([TAG, TAG], f32, tag=f"ts{_rep}")
                nc.sync.dma_start(tsb[:], trans[:, :])
                tag_all = cpool.tile([128, NCH * BPC], f32, tag=f"tg{_rep}")
                prev_all = cpool.tile([128, NCH * BPC], f32, tag=f"pv{_rep}")
                for c in range(NCH):
                    nc.sync.dma_start(
                        tag_all[:, c * BPC:(c + 1) * BPC],
                        tagsf[:, bass.ts(c, CH)].rearrange("b t -> t b"))
                    nc.sync.dma_start(
                        prev_all[:, c * BPC:(c + 1) * BPC],
                        prevf[:, bass.ts(c, CH)].rearrange("b t -> t b"))
                endsb = cpool.tile([BPC, 1], f32, tag=f"en{_rep}")
                nc.sync.dma_start(endsb[:], endf[:, :])
                ksb = cpool.tile([1, BPC], f32, tag=f"ks{_rep}")
                nc.sync.dma_start(ksb[:], ksum_d[:, :])
                fbuf = bigpool.tile([128, NCH * BPC * TAG], f32,
                                    tag="fb", name="fb")
                fb3 = fbuf[:].rearrange("p (c b g) -> p c b g", c=NCH, b=BPC)
                for c in range(NCH):
                    nc.sync.dma_start(
                        fb3[:, c, :, :],
                        feats[:, bass.ts(c, CH), :].rearrange(
                            "b t g -> t b g"))

                # ---------- stationary + init (exp of host log consts) -----
                vdt = bf16 if wdt == "bf16" else f32
                Wfb = cpool.tile([128, 128], vdt, tag=f"W{_rep}")
                nc.scalar.activation(Wfb[:], wl[:], AF.Exp)
                initcol = cpool.tile([128, 1], f32, tag=f"ic{_rep}")
                nc.scalar.activation(initcol[:], il[:], AF.Exp)

                # ---------- exp(feats) in place, first-needed block first ---
                ef_views = []
                KB = HALF // NEB
                for i in range(NEB):
                    nc.scalar.activation(efts[i][:], efts[i][:], AF.Exp)
                    ef_views.append(efts[i][:].rearrange(
                        "p (t b) -> p t b", b=BPC))

                def ef_at(k):
                    return ef_views[k // KB][:, k % KB, :]

                # ---------- gold one-hots + emission terms ----------
                # gp=True: on GpSimd, issued before the scan (its queue is
                # otherwise idle). gp=False: on DVE, issued after the scan
                # TTs so they do not delay the chain.
                if gp == "gp":
                    emitbuf = cpool.tile([1, NCH * BPC], f32,
                                         tag=f"em{_rep}")
                else:
                    emitbuf = cpool.tile([128, NCH * BPC], f32,
                                         tag=f"em{_rep}")
                oTs = {}
                oPs = {}
                oh_end = cpool.tile([BPC, TAG], f32, tag=f"oe{_rep}")

                oT_all = None
                emit_reduce = []

                def do_onehots_big():
                    nonlocal oT_all
                    NC_ = NCH * BPC
                    oT_all = bigpool.tile([128, NC_ * TAG], f32,
                                          tag="oTa",
                                          name="oTa")
                    oP_all = bigpool.tile([128, NC_ * TAG], f32,
                                          tag="oPa",
                                          name="oPa")
                    tag_b = tag_all[:].rearrange(
                        "p (a o) -> p a o", o=1).broadcast_to([128, NC_, TAG])
                    prev_b = prev_all[:].rearrange(
                        "p (a o) -> p a o", o=1).broadcast_to([128, NC_, TAG])
                    i3 = iotarep[:].rearrange("p (a g) -> p a g", g=TAG)
                    eng = nc.vector
                    eng.tensor_tensor(
                        oT_all[:].rearrange("p (a g) -> p a g", g=TAG),
                        i3, tag_b, op=ALU.is_equal)
                    eng.tensor_tensor(
                        oP_all[:].rearrange("p (a g) -> p a g", g=TAG),
                        i3, prev_b, op=ALU.is_equal)
                    em_all = bigpool.tile([128, NC_ * TAG], f32,
                                          tag="ema",
                                          name="ema")
                    eng.tensor_tensor(em_all[:], fbuf[:], oT_all[:],
                                      op=ALU.mult)
                    if gp in ("big2", "big3"):
                        emit_reduce.append(em_all)
                    else:
                        nc.vector.tensor_reduce(
                            emitbuf[:],
                            em_all[:].rearrange("p (a g) -> p a g", g=TAG),
                            axis=AX.X, op=ALU.add)
                    for col in range(NC_):
                        oTs[col] = oT_all[:, col * TAG:(col + 1) * TAG]
                        oPs[col] = oP_all[:, col * TAG:(col + 1) * TAG]
                    nc.vector.tensor_scalar(oh_end[:],
                                            iota_row_f[:BPC, :TAG],
                                            endsb[:], None,
                                            op0=ALU.is_equal)

                def do_onehots():
                    if gp in ("big", "big2", "big3"):
                        do_onehots_big()
                        return
                    eng = nc.gpsimd if gp == "gp" else nc.vector
                    for c in range(NCH):
                        for b in range(BPC):
                            col = c * BPC + b
                            oT = ohpool.tile([128, TAG], f32, tag="oT",
                                             name=f"oT{_rep}_{col}")
                            eng.tensor_scalar(
                                oT[:], iota_row_f[:, :TAG],
                                tag_all[:, col:col + 1], None,
                                op0=ALU.is_equal)
                            oP = ohpool.tile([128, TAG], f32, tag="oP",
                                             name=f"oP{_rep}_{col}")
                            eng.tensor_scalar(
                                oP[:], iota_row_f[:, :TAG],
                                prev_all[:, col:col + 1], None,
                                op0=ALU.is_equal)
                            em = ohpool.tile([128, TAG], f32, tag="em")
                            eng.tensor_tensor(em[:], fb3[:, c, b, :],
                                              oT[:], op=ALU.mult)
                            if gp == "gp":
                                eng.tensor_reduce(emitbuf[:, col:col + 1],
                                                  em[:], axis=AX.XYZWC,
                                                  op=ALU.add)
                            else:
                                eng.tensor_reduce(emitbuf[:, col:col + 1],
                                                  em[:], axis=AX.X,
                                                  op=ALU.add)
                            oTs[col] = oT
                            oPs[col] = oP
                    eng.tensor_scalar(oh_end[:], iota_row_f[:BPC, :TAG],
                                      endsb[:], None, op0=ALU.is_equal)

                if gp == "gp" and not no_gold:
                    do_onehots()

                # ---------- rescale bookkeeping ----------
                FINAL_LAZY = 2
                final_k = HALF - 1 - FINAL_LAZY
                resc = {}
                for k in range(1, HALF):
                    if (k % rr == rr - 1 and k + lazy <= HALF - 1
                            and k + lazy != final_k + FINAL_LAZY
                            and k != final_k):
                        resc[k] = lazy
                resc[final_k] = FINAL_LAZY
                resc_steps = sorted(resc)
                if no_resc:
                    resc = {}
                    resc_steps = []
                nresc = len(resc_steps)
                lnbuf0 = cpool.tile([1, max(nresc, 1) * BPC], f32,
                                    tag=f"lb0{_rep}")
                lnbuf1 = cpool.tile([1, max(nresc, 1) * BPC], f32,
                                    tag=f"lb1{_rep}")
                if nresc == 0:
                    nc.vector.memset(lnbuf0[:], 0.0)
                    nc.vector.memset(lnbuf1[:], 0.0)

                if gp in ("big", "big2", "big3") and not no_gold:
                    do_onehots()

                # count matmuls as thunks; big2 interleaves them into PE
                # idle windows at rescale points (stationary reloads there
                # anyway); remaining thunks drain at the tail.
                cnt_thunks = []
                if gp in ("big2", "big3") and not no_gold:
                    count_ps_i = ps_cnt.tile([TAG, TAG], f32)

                    def _mk(col, first):
                        def t():
                            nc.tensor.matmul(count_ps_i[:], oPs[col][:],
                                             oTs[col][:], start=first,
                                             stop=False,
                                             skip_group_check=True)
                        return t
                    for col in range(NCH * BPC):
                        cnt_thunks.append(_mk(col, col == 0))

                # ---------- fused forward+backward scan ----------
                v = vpool.tile([128, BPC], vdt, tag="v")
                nc.vector.tensor_scalar(v[:], ef_at(0), initcol[:],
                                        None, op0=ALU.mult)
                folds = {}
                pending = None
                for k in range(1, HALF):
                    s_ps = ps_s.tile([128, BPC], f32, tag="s")
                    nc.tensor.matmul(s_ps[:], Wfb[:], v[:], start=True,
                                     stop=True)
                    src_ap = folds.pop(k, None)
                    if src_ap is None:
                        src_ap = ef_at(k)
                    else:
                        src_ap = src_ap[:]
                    v2 = vpool.tile([128, BPC], vdt, tag="v")
                    nc.vector.tensor_tensor(v2[:], src_ap, s_ps[:],
                                            op=ALU.mult)
                    v = v2
                    if gp == "big3" and not no_gold:
                        if 100 <= k and cnt_thunks:
                            cnt_thunks.pop(0)()
                        if k in (150, 170, 190, 210) and emit_reduce:
                            i = (150, 170, 190, 210).index(k)
                            em_all = emit_reduce[0]
                            nc.vector.tensor_reduce(
                                emitbuf[:, i * 8:(i + 1) * 8],
                                em_all[:, i * 400:(i + 1) * 400].rearrange(
                                    "p (a g) -> p a g", g=TAG),
                                axis=AX.X, op=ALU.add)
                    elif pending is not None and k > 64 and cnt_thunks:
                        for _ in range(3):
                            if cnt_thunks:
                                cnt_thunks.pop(0)()
                    if pending is not None:
                        # broadcast matmuls issued AFTER this step's chain
                        # matmul so PE never stalls waiting on rm
                        rm0, rm1, tgt = pending
                        rb_ps = ps_m.tile([128, BPC], f32, tag="m")
                        nc.tensor.matmul(rb_ps[:], IndF[:], rm0[:],
                                         start=True, stop=False,
                                         skip_group_check=True)
                        nc.tensor.matmul(rb_ps[:], IndB[:], rm1[:],
                                         start=False, stop=True,
                                         skip_group_check=True)
                        emod = spool.tile([128, BPC], f32, tag="emod")
                        if resc_eng == "pool":
                            rbs = spool.tile([128, BPC], f32, tag="rbs")
                            nc.scalar.copy(rbs[:], rb_ps[:])
                            nc.gpsimd.tensor_tensor(emod[:], ef_at(tgt),
                                                    rbs[:], op=ALU.mult)
                        else:
                            nc.vector.tensor_tensor(emod[:], ef_at(tgt),
                                                    rb_ps[:], op=ALU.mult)
                        folds[tgt] = emod
                        pending = None
                    if k in resc:
                        # per-half column sums of v_{k-1} from rows 32 / 96
                        ri = resc_steps.index(k)
                        rm0 = spool.tile([1, BPC], f32, tag="rm0")
                        rm1 = spool.tile([1, BPC], f32, tag="rm1")
                        nc.vector.reciprocal(rm0[:], s_ps[32:33, :])
                        nc.vector.reciprocal(rm1[:], s_ps[96:97, :])
                        nc.scalar.activation(
                            lnbuf0[:, ri * BPC:(ri + 1) * BPC],
                            s_ps[32:33, :], AF.Ln)
                        nc.scalar.activation(
                            lnbuf1[:, ri * BPC:(ri + 1) * BPC],
                            s_ps[96:97, :], AF.Ln)
                        pending = (rm0, rm1, k + resc[k])

                # ---------- terminal combine ----------
                s_ps = ps_s.tile([128, BPC], f32, tag="s")
                nc.tensor.matmul(s_ps[:], Wfb[:], v[:], start=True, stop=True)
                zt = spool.tile([64, BPC], f32, tag="zt")
                nc.vector.tensor_tensor(zt[:], v[0:64, :], s_ps[64:128, :],
                                        op=ALU.mult)
                z_ps = ps_z.tile([1, BPC], f32, tag="z")
                nc.tensor.matmul(z_ps[:], ones64[:], zt[:], start=True,
                                 stop=True)
                lnz = spool.tile([1, BPC], f32, tag="lnz")
                nc.scalar.activation(lnz[:], z_ps[:], AF.Ln)
                Csb0 = spool.tile([1, BPC], f32, tag="cs0")
                nc.vector.tensor_reduce(
                    Csb0[:], lnbuf0[:].rearrange("p (r b) -> p b r", b=BPC),
                    axis=AX.X, op=ALU.add)
                Csb1 = spool.tile([1, BPC], f32, tag="cs1")
                nc.vector.tensor_reduce(
                    Csb1[:], lnbuf1[:].rearrange("p (r b) -> p b r", b=BPC),
                    axis=AX.X, op=ALU.add)
                fwd = cpool.tile([1, BPC], f32, tag=f"fw{_rep}")
                nc.vector.tensor_add(fwd[:], lnz[:], Csb0[:])
                nc.vector.tensor_add(fwd[:], fwd[:], Csb1[:])
                nc.vector.tensor_add(fwd[:], fwd[:], ksb[:])
                if osb_prev is not None:
                    # inert data dependency to serialize reps
                    nc.vector.tensor_scalar(fwd[:, 0:1], osb_prev[:, 0:1],
                                            0.0, fwd[:, 0:1],
                                            op0=ALU.mult, op1=ALU.add)

                # ---------- gold: count matmuls (PE tail) ----------
                if gp == "dve" and not no_gold:
                    do_onehots()
                if gp in ("big2", "big3") and not no_gold:
                    while cnt_thunks:
                        cnt_thunks.pop(0)()
                    count_ps = count_ps_i
                    nc.tensor.matmul(count_ps[:], oh_end[:], oh_stop[:],
                                     start=False, stop=True,
                                     skip_group_check=True)
                    if gp == "big2":
                        for em_all in emit_reduce:
                            nc.vector.tensor_reduce(
                                emitbuf[:],
                                em_all[:].rearrange("p (a g) -> p a g",
                                                    g=TAG),
                                axis=AX.X, op=ALU.add)
                else:
                    count_ps = ps_cnt.tile([TAG, TAG], f32)
                if no_gold:
                    nc.vector.memset(count_ps[:], 0.0)
                first = True
                if not no_gold and gp != "big2":
                    for c in range(NCH):
                        for b in range(BPC):
                            col = c * BPC + b
                            oPa = oPs[col]
                            oTa = oTs[col]
                            if hasattr(oPa, 'tile'):
                                pass
                            try:
                                oPa = oPa[:]
                                oTa = oTa[:]
                            except Exception:
                                pass
                            nc.tensor.matmul(count_ps[:], oPa, oTa,
                                             start=first, stop=False,
                                             skip_group_check=True)
                            first = False
                    nc.tensor.matmul(count_ps[:], oh_end[:], oh_stop[:],
                                     start=False, stop=True,
                                     skip_group_check=True)
                tmul = spool.tile([TAG, TAG], f32, tag="tmul")
                nc.vector.tensor_tensor(tmul[:], tsb[:], count_ps[:],
                                        op=ALU.mult)
                tred = spool.tile([TAG, 1], f32, tag="tred")
                nc.vector.tensor_reduce(tred[:], tmul[:], axis=AX.X,
                                        op=ALU.add)
                gt_ps = ps_z.tile([1, 1], f32, tag="z")
                nc.tensor.matmul(gt_ps[:], ones50[:], tred[:], start=True,
                                 stop=True)
                gemit = spool.tile([1, 1], f32, tag="gem")
                if no_gold:
                    nc.vector.memset(gemit[:], 0.0)
                elif gp == "gp":
                    nc.vector.tensor_reduce(gemit[:], emitbuf[:], axis=AX.X,
                                            op=ALU.add)
                else:
                    ep_ps = ps_z.tile([1, NCH * BPC], f32, tag="z")
                    nc.tensor.matmul(ep_ps[:], ones128[:], emitbuf[:],
                                     start=True, stop=True)
                    nc.vector.tensor_reduce(gemit[:], ep_ps[:], axis=AX.X,
                                            op=ALU.add)

                # ---------- assemble output ----------
                osb = cpool.tile([1, 16], f32, tag=f"osb{_rep}",
                                 name=f"osb{_rep}")
                nc.vector.memset(osb[:], 0.0)
                nc.vector.tensor_copy(osb[:, 0:BPC], fwd[:])
                nc.vector.tensor_copy(osb[:, 8:9], gemit[:])
                nc.vector.tensor_copy(osb[:, 9:10], gt_ps[:])
                nc.sync.dma_start(out[:, :], osb[:])
                osb_prev = osb

    nc.compile()
    return nc, "out"


def _numpy_reference(feats, mask, tags, transitions):
    maskf = mask.astype(np.float64)
    f = feats.astype(np.float64)
    T = transitions.astype(np.float64)
    b, s, t = f.shape
    part = f[:, 0, :] + T[START][None, :]
    for ti in range(1, s):
        cur = part[:, :, None] + T[None, :, :] + f[:, ti, None, :]
        m = cur.max(axis=1)
        cur = m + np.log(np.exp(cur - m[:, None, :]).sum(axis=1))
        part = np.where(mask[:, ti][:, None].astype(bool), cur, part)
    term = part[:, :, None] + T[None, :, :]
    m = term.max(axis=1)
    term = m + np.log(np.exp(term - m[:, None, :]).sum(axis=1))
    forward = term[:, STOP].sum()
    prev = np.concatenate([np.full((b, 1), START, dtype=tags.dtype),
                           tags[:, :-1]], axis=1)
    emit = np.take_along_axis(f, tags[..., None], axis=2)[..., 0]
    tr = T[prev, tags]
    tg = ((emit + tr) * maskf).sum()
    lengths = mask.astype(np.int64).sum(axis=1)
    end_ids = np.take_along_axis(tags, (lengths - 1)[:, None], axis=1)[:, 0]
    gold = tg + T[end_ids, STOP].sum()
    return np.array(forward - gold, dtype=np.float32)


def kernel(feats, mask, tags, transitions):
    global _COMPILED, LAST_RESULTS, LAST_IN_MAPS
    feats = np.asarray(feats, dtype=np.float32)
    mask = np.asarray(mask)
    tags = np.asarray(tags)
    transitions = np.asarray(transitions, dtype=np.float32)

    if not np.all(mask == 1):
        return _numpy_reference(feats, np.asarray(mask, dtype=np.int64),
                                np.asarray(tags, dtype=np.int64), transitions)

    if 1 not in _COMPILED:
        _COMPILED[1] = _build(reps=1)
    nc, out_name = _COMPILED[1]

    tags_i = tags.astype(np.int64)
    prev = np.concatenate(
        [np.full((B, 1), START, dtype=np.int64), tags_i[:, :-1]], axis=1)
    lengths = mask.astype(np.int64).sum(axis=1)
    end_ids = np.take_along_axis(tags_i, (lengths - 1)[:, None], axis=1)[:, 0]

    tagsf = tags_i.astype(np.float32)
    prevf = prev.astype(np.float32)
    endf = end_ids.astype(np.float32).reshape(B, 1)
    wlog, initlog, indf, indb = _host_consts(transitions)

    in_maps = []
    for c in range(NCORES):
        sl = slice(c * BPC, (c + 1) * BPC)
        fshard = feats[sl]                       # (8, 512, 50)
        kap = fshard.max(axis=2)                 # (8, 512) host kappa shift
        fsh = fshard - kap[:, :, None]           # emissions <= 1 in exp dom
        ksum = kap.sum(axis=1).astype(np.float32).reshape(1, BPC)
        fr = fsh.transpose(2, 1, 0)              # (50, 512, 8)
        efraw = np.full((128, HALF, BPC), NEG, dtype=np.float32)
        efraw[_RMAP] = fr[:, :HALF, :]           # fwd: t = k
        efraw[64 + _RMAP] = fr[:, ::-1, :][:, :HALF, :]  # bwd: t = 511-k
        in_maps.append({
            "ksum": ksum,
            "efraw": np.ascontiguousarray(efraw.reshape(128, HALF * BPC)),
            "wlog": wlog,
            "initlog": initlog,
            "indf": indf,
            "indb": indb,
            "feats": np.ascontiguousarray(fshard),
            "tagsf": np.ascontiguousarray(tagsf[sl]),
            "prevf": np.ascontiguousarray(prevf[sl]),
            "endf": np.ascontiguousarray(endf[sl]),
            "trans": transitions,
        })

    from concourse import bass_utils
    res = bass_utils.run_bass_kernel_spmd(nc, in_maps,
                                          core_ids=list(range(NCORES)))
    LAST_RESULTS = res
    LAST_IN_MAPS = in_maps

    total = 0.0
    for c in range(NCORES):
        o = res.results[c][out_name].astype(np.float64)[0]
        total += o[0:BPC].sum() - o[8] - o[9]
    return np.array(total, dtype=np.float32)


# revision 7
# speedup vs baseline: 1.5855x; 1.3196x over previous
"""CRF negative-log-likelihood loss kernel for Trainium2 (8 NeuronCores), v2.

Data-parallel over batch (64 seqs -> 8 cores x 8 seqs). The forward
(log-partition) scan runs in the exp domain as a FUSED forward+backward
meet-in-the-middle chain, halving sequential depth 511 -> 255:

    state u_k = [alpha_k ; gamma_k]   (128 partitions x 8 seqs, bf16)
    alpha_k = e_k (.) (T^T alpha_{k-1})          (fwd, rows 0:64)
    gamma_k = e_{511-k} (.) (T gamma_{k-1})      (bwd, rows 64:128)
    Z_b = sum_i alpha_255[i,b] * (T gamma_255)[i,b]

Partition layout per half (32-aligned access rule): tags 0-31 at rows
0-31, the half's column-sum row at 32 (fed by a ones column in the
stationary -> per-step sums for free), tags 32-49 at rows 34-51, rest
padded with exp(-30000)=0. One [128,128] bf16 stationary (blockdiag
expT / expT^T, host-prepermuted in log space) gives ONE matmul + ONE
DVE multiply per superstep. Emissions are unshifted; fp32 range kept
safe by per-column rescales every R=5 steps, folded into the emission
buffer LAZY=2 steps later (off the critical path). Validated on the
actual input distribution: max |ln m| ~ 26 << 44 (Ln table range).

Gold score on-device: one-hot/emission work on GpSimd (idle during the
scan), (prev,tag) count matmuls on PE after the scan.
"""

import numpy as np

TAG = 50
START = TAG - 2
STOP = TAG - 1
B, S = 64, 512
NCORES = 8
BPC = B // NCORES  # sequences per core
HALF = S // 2      # supersteps
NCH = 4
CH = S // NCH
R = 11             # rescale period (host kappa shift: max |ln m| = 34.4 < 44)
LAZY = 4           # rescale factor applied LAZY steps later
NEG = -30000.0     # exp(NEG) == 0 padding

# within-half row of tag t (sum row sits at 32)
_RMAP = np.array([t if t < 32 else t + 2 for t in range(TAG)])

_COMPILED = {}
LAST_RESULTS = None
LAST_IN_MAPS = None


def _host_consts(transitions):
    """Host-prepermuted log-space stationary + init column + indicators."""
    T = transitions.astype(np.float32)
    wlog = np.full((128, 128), NEG, dtype=np.float32)
    r = _RMAP
    # fwd block: out row r(j) = sum_i T[i,j] v[r(i)]
    wlog[np.ix_(r, r)] = T
    # bwd block: out row 64+r(i) = sum_j T[i,j] v[64+r(j)]
    wlog[np.ix_(64 + r, 64 + r)] = T.T
    # per-half sum columns (exp(0)=1 weights)
    wlog[r, 32] = 0.0
    wlog[64 + r, 96] = 0.0
    initlog = np.full((128, 1), NEG, dtype=np.float32)
    initlog[r, 0] = T[START, :]
    initlog[64 + r, 0] = T[:, STOP]
    indf = np.zeros((1, 128), dtype=np.float32)
    indf[0, r] = 1.0
    indb = np.zeros((1, 128), dtype=np.float32)
    indb[0, 64 + r] = 1.0
    return wlog, initlog, indf, indb


def _build(reps=1, gp="big", wdt="bf16", no_gold=False, no_resc=False,
           ps_bufs=4, v_bufs=3, lazy=LAZY, rr=R, resc_eng="dve"):
    import concourse.bass as bass
    import concourse.bacc as bacc
    import concourse.tile as tile
    from concourse import mybir

    f32 = mybir.dt.float32
    bf16 = mybir.dt.bfloat16
    i32 = mybir.dt.int32
    AF = mybir.ActivationFunctionType
    ALU = mybir.AluOpType
    AX = mybir.AxisListType

    nc = bacc.Bacc("TRN2", target_bir_lowering=False, debug=False,
                   enable_asserts=False, num_devices=NCORES)

    # efraw: fused-layout raw feats [128, 256*8], col = k*8+b:
    #   row rmap(g)    = feats[b, k, g]      (fwd)
    #   row 64+rmap(g) = feats[b, 511-k, g]  (bwd), pad rows = NEG
    efraw_d = nc.dram_tensor("efraw", [128, HALF * BPC], f32,
                             kind="ExternalInput")
    wlog_d = nc.dram_tensor("wlog", [128, 128], f32, kind="ExternalInput")
    initlog_d = nc.dram_tensor("initlog", [128, 1], f32,
                               kind="ExternalInput")
    indf_d = nc.dram_tensor("indf", [1, 128], f32, kind="ExternalInput")
    indb_d = nc.dram_tensor("indb", [1, 128], f32, kind="ExternalInput")
    feats = nc.dram_tensor("feats", [BPC, S, TAG], f32, kind="ExternalInput")
    tagsf = nc.dram_tensor("tagsf", [BPC, S], f32, kind="ExternalInput")
    prevf = nc.dram_tensor("prevf", [BPC, S], f32, kind="ExternalInput")
    endf = nc.dram_tensor("endf", [BPC, 1], f32, kind="ExternalInput")
    ksum_d = nc.dram_tensor("ksum", [1, BPC], f32, kind="ExternalInput")
    trans = nc.dram_tensor("trans", [TAG, TAG], f32, kind="ExternalInput")
    out = nc.dram_tensor("out", [1, 16], f32, kind="ExternalOutput")

    with tile.TileContext(nc) as tc:
        with tc.tile_pool(name="const", bufs=1) as cpool, \
             tc.tile_pool(name="big", bufs=1) as bigpool, \
             tc.tile_pool(name="oh", bufs=6) as ohpool, \
             tc.tile_pool(name="small", bufs=4) as spool, \
             tc.tile_pool(name="v", bufs=v_bufs) as vpool, \
             tc.tile_pool(name="ps_s", bufs=ps_bufs, space="PSUM") as ps_s, \
             tc.tile_pool(name="ps_m", bufs=2, space="PSUM") as ps_m, \
             tc.tile_pool(name="ps_cnt", bufs=1, space="PSUM") as ps_cnt, \
             tc.tile_pool(name="ps_z", bufs=1, space="PSUM") as ps_z:

            # ---------- constants ----------
            iota_row_i = cpool.tile([128, 128], i32)
            nc.gpsimd.iota(iota_row_i[:], pattern=[[1, 128]], base=0,
                           channel_multiplier=0)
            iota_row_f = cpool.tile([128, 128], f32)
            nc.vector.tensor_copy(iota_row_f[:], iota_row_i[:])
            ones64 = cpool.tile([64, 1], f32)
            nc.vector.memset(ones64[:], 1.0)
            ones50 = cpool.tile([TAG, 1], f32)
            nc.vector.memset(ones50[:], 1.0)
            ones128 = cpool.tile([128, 1], f32)
            nc.vector.memset(ones128[:], 1.0)
            oh_stop = cpool.tile([BPC, TAG], f32)
            nc.vector.tensor_scalar(oh_stop[:], iota_row_f[:BPC, :TAG],
                                    float(STOP), None, op0=ALU.is_equal)
            iotarep_i = cpool.tile([128, NCH * BPC * TAG], i32)
            nc.gpsimd.iota(iotarep_i[:], pattern=[[0, NCH * BPC], [1, TAG]],
                           base=0, channel_multiplier=0)
            iotarep = cpool.tile([128, NCH * BPC * TAG], f32)
            nc.vector.tensor_copy(iotarep[:], iotarep_i[:])

            osb_prev = None
            for _rep in range(reps):
                # ---------- input DMAs ----------
                # tiny chain-critical inputs FIRST (SP queue is in-order):
                # the stationary + init column must not wait behind the
                # 256KB emission blocks.
                wl = cpool.tile([116, 116], f32, tag=f"wl{_rep}")
                nc.sync.dma_start(wl[:], wlog_d[0:116, 0:116])
                il = cpool.tile([116, 1], f32, tag=f"il{_rep}")
                nc.sync.dma_start(il[:], initlog_d[0:116, :])
                NEB = 4
                EB = HALF * BPC // NEB
                efts = []
                for i in range(NEB):
                    eft = bigpool.tile([116, EB], f32, tag=f"eq{i}",
                                       name=f"eq{i}")
                    nc.sync.dma_start(eft[:],
                                      efraw_d[0:116, i * EB:(i + 1) * EB])
                    efts.append(eft)
                IndF = cpool.tile([1, 128], f32, tag=f"if{_rep}")
                nc.sync.dma_start(IndF[:], indf_d[:, :])
                IndB = cpool.tile([1, 128], f32, tag=f"ib{_rep}")
                nc.sync.dma_start(IndB[:], indb_d[:, :])
                tsb = cpool.tile([TAG, TAG], f32, tag=f"ts{_rep}")
                nc.sync.dma_start(tsb[:], trans[:, :])
                tag_all = cpool.tile([128, NCH * BPC], f32, tag=f"tg{_rep}")
                prev_all = cpool.tile([128, NCH * BPC], f32, tag=f"pv{_rep}")
                for c in range(NCH):
                    nc.sync.dma_start(
                        tag_all[:, c * BPC:(c + 1) * BPC],
                        tagsf[:, bass.ts(c, CH)].rearrange("b t -> t b"))
                    nc.sync.dma_start(
                        prev_all[:, c * BPC:(c + 1) * BPC],
                        prevf[:, bass.ts(c, CH)].rearrange("b t -> t b"))
                endsb = cpool.tile([BPC, 1], f32, tag=f"en{_rep}")
                nc.sync.dma_start(endsb[:], endf[:, :])
                ksb = cpool.tile([1, BPC], f32, tag=f"ks{_rep}")
                nc.sync.dma_start(ksb[:], ksum_d[:, :])
                fbuf = bigpool.tile([128, NCH * BPC * TAG], f32,
                                    tag="fb", name="fb")
                fb3 = fbuf[:].rearrange("p (c b g) -> p c b g", c=NCH, b=BPC)
                for c in range(NCH):
                    nc.sync.dma_start(
                        fb3[:, c, :, :],
                        feats[:, bass.ts(c, CH), :].rearrange(
                            "b t g -> t b g"))

                # ---------- stationary + init (exp of host log consts) -----
                vdt = bf16 if wdt == "bf16" else f32
                Wfb = cpool.tile([116, 116], vdt, tag=f"W{_rep}")
                nc.scalar.activation(Wfb[:], wl[:], AF.Exp)
                initcol = cpool.tile([116, 1], f32, tag=f"ic{_rep}")
                nc.scalar.activation(initcol[:], il[:], AF.Exp)

                # ---------- exp(feats) in place, first-needed block first ---
                ef_views = []
                KB = HALF // NEB
                for i in range(NEB):
                    nc.scalar.activation(efts[i][:], efts[i][:], AF.Exp)
                    ef_views.append(efts[i][:].rearrange(
                        "p (t b) -> p t b", b=BPC))

                def ef_at(k):
                    return ef_views[k // KB][:, k % KB, :]

                # ---------- gold one-hots + emission terms ----------
                # gp=True: on GpSimd, issued before the scan (its queue is
                # otherwise idle). gp=False: on DVE, issued after the scan
                # TTs so they do not delay the chain.
                if gp == "gp":
                    emitbuf = cpool.tile([1, NCH * BPC], f32,
                                         tag=f"em{_rep}")
                else:
                    emitbuf = cpool.tile([128, NCH * BPC], f32,
                                         tag=f"em{_rep}")
                oTs = {}
                oPs = {}
                oh_end = cpool.tile([BPC, TAG], f32, tag=f"oe{_rep}")

                oT_all = None
                emit_reduce = []

                def do_onehots_big():
                    nonlocal oT_all
                    NC_ = NCH * BPC
                    oT_all = bigpool.tile([128, NC_ * TAG], f32,
                                          tag="oTa",
                                          name="oTa")
                    oP_all = bigpool.tile([128, NC_ * TAG], f32,
                                          tag="oPa",
                                          name="oPa")
                    tag_b = tag_all[:].rearrange(
                        "p (a o) -> p a o", o=1).broadcast_to([128, NC_, TAG])
                    prev_b = prev_all[:].rearrange(
                        "p (a o) -> p a o", o=1).broadcast_to([128, NC_, TAG])
                    i3 = iotarep[:].rearrange("p (a g) -> p a g", g=TAG)
                    eng = nc.vector
                    eng.tensor_tensor(
                        oT_all[:].rearrange("p (a g) -> p a g", g=TAG),
                        i3, tag_b, op=ALU.is_equal)
                    eng.tensor_tensor(
                        oP_all[:].rearrange("p (a g) -> p a g", g=TAG),
                        i3, prev_b, op=ALU.is_equal)
                    em_all = bigpool.tile([128, NC_ * TAG], f32,
                                          tag="ema",
                                          name="ema")
                    eng.tensor_tensor(em_all[:], fbuf[:], oT_all[:],
                                      op=ALU.mult)
                    if gp in ("big2", "big3"):
                        emit_reduce.append(em_all)
                    else:
                        nc.vector.tensor_reduce(
                            emitbuf[:],
                            em_all[:].rearrange("p (a g) -> p a g", g=TAG),
                            axis=AX.X, op=ALU.add)
                    for col in range(NC_):
                        oTs[col] = oT_all[:, col * TAG:(col + 1) * TAG]
                        oPs[col] = oP_all[:, col * TAG:(col + 1) * TAG]
                    nc.vector.tensor_scalar(oh_end[:],
                                            iota_row_f[:BPC, :TAG],
                                            endsb[:], None,
                                            op0=ALU.is_equal)

                def do_onehots():
                    if gp in ("big", "big2", "big3"):
                        do_onehots_big()
                        return
                    eng = nc.gpsimd if gp == "gp" else nc.vector
                    for c in range(NCH):
                        for b in range(BPC):
                            col = c * BPC + b
                            oT = ohpool.tile([128, TAG], f32, tag="oT",
                                             name=f"oT{_rep}_{col}")
                            eng.tensor_scalar(
                                oT[:], iota_row_f[:, :TAG],
                                tag_all[:, col:col + 1], None,
                                op0=ALU.is_equal)
                            oP = ohpool.tile([128, TAG], f32, tag="oP",
                                             name=f"oP{_rep}_{col}")
                            eng.tensor_scalar(
                                oP[:], iota_row_f[:, :TAG],
                                prev_all[:, col:col + 1], None,
                                op0=ALU.is_equal)
                            em = ohpool.tile([128, TAG], f32, tag="em")
                            eng.tensor_tensor(em[:], fb3[:, c, b, :],
                                              oT[:], op=ALU.mult)
                            if gp == "gp":
                                eng.tensor_reduce(emitbuf[:, col:col + 1],
                                                  em[:], axis=AX.XYZWC,
                                                  op=ALU.add)
                            else:
                                eng.tensor_reduce(emitbuf[:, col:col + 1],
                                                  em[:], axis=AX.X,
                                                  op=ALU.add)
                            oTs[col] = oT
                            oPs[col] = oP
                    eng.tensor_scalar(oh_end[:], iota_row_f[:BPC, :TAG],
                                      endsb[:], None, op0=ALU.is_equal)

                if gp == "gp" and not no_gold:
                    do_onehots()

                # ---------- rescale bookkeeping ----------
                FINAL_LAZY = 2
                final_k = HALF - 1 - FINAL_LAZY
                resc = {}
                for k in range(1, HALF):
                    if (k % rr == rr - 1 and k + lazy <= HALF - 1
                            and k + lazy != final_k + FINAL_LAZY
                            and k != final_k):
                        resc[k] = lazy
                resc[final_k] = FINAL_LAZY
                resc_steps = sorted(resc)
                if no_resc:
                    resc = {}
                    resc_steps = []
                nresc = len(resc_steps)
                lnbuf0 = cpool.tile([1, max(nresc, 1) * BPC], f32,
                                    tag=f"lb0{_rep}")
                lnbuf1 = cpool.tile([1, max(nresc, 1) * BPC], f32,
                                    tag=f"lb1{_rep}")
                if nresc == 0:
                    nc.vector.memset(lnbuf0[:], 0.0)
                    nc.vector.memset(lnbuf1[:], 0.0)

                if gp in ("big", "big2", "big3") and not no_gold:
                    do_onehots()

                # count matmuls as thunks; big2 interleaves them into PE
                # idle windows at rescale points (stationary reloads there
                # anyway); remaining thunks drain at the tail.
                cnt_thunks = []
                if gp in ("big2", "big3") and not no_gold:
                    count_ps_i = ps_cnt.tile([TAG, TAG], f32)

                    def _mk(col, first):
                        def t():
                            nc.tensor.matmul(count_ps_i[:], oPs[col][:],
                                             oTs[col][:], start=first,
                                             stop=False,
                                             skip_group_check=True)
                        return t
                    for col in range(NCH * BPC):
                        cnt_thunks.append(_mk(col, col == 0))

                # ---------- fused forward+backward scan ----------
                v = vpool.tile([116, BPC], vdt, tag="v")
                nc.vector.tensor_scalar(v[:], ef_at(0), initcol[:],
                                        None, op0=ALU.mult)
                folds = {}
                pending = None
                for k in range(1, HALF):
                    s_ps = ps_s.tile([116, BPC], f32, tag="s")
                    nc.tensor.matmul(s_ps[:], Wfb[:], v[:], start=True,
                                     stop=True)
                    src_ap = folds.pop(k, None)
                    if src_ap is None:
                        src_ap = ef_at(k)
                    else:
                        src_ap = src_ap[:]
                    v2 = vpool.tile([116, BPC], vdt, tag="v")
                    nc.vector.tensor_tensor(v2[:], src_ap, s_ps[:],
                                            op=ALU.mult)
                    v = v2
                    if gp == "big3" and not no_gold:
                        if 100 <= k and cnt_thunks:
                            cnt_thunks.pop(0)()
                        if k in (150, 170, 190, 210) and emit_reduce:
                            i = (150, 170, 190, 210).index(k)
                            em_all = emit_reduce[0]
                            nc.vector.tensor_reduce(
                                emitbuf[:, i * 8:(i + 1) * 8],
                                em_all[:, i * 400:(i + 1) * 400].rearrange(
                                    "p (a g) -> p a g", g=TAG),
                                axis=AX.X, op=ALU.add)
                    elif pending is not None and k > 64 and cnt_thunks:
                        for _ in range(3):
                            if cnt_thunks:
                                cnt_thunks.pop(0)()
                    if pending is not None:
                        # broadcast matmuls issued AFTER this step's chain
                        # matmul so PE never stalls waiting on rm
                        rm0, rm1, tgt = pending
                        rb_ps = ps_m.tile([116, BPC], f32, tag="m")
                        nc.tensor.matmul(rb_ps[:], IndF[:, 0:116], rm0[:],
                                         start=True, stop=False,
                                         skip_group_check=True)
                        nc.tensor.matmul(rb_ps[:], IndB[:, 0:116], rm1[:],
                                         start=False, stop=True,
                                         skip_group_check=True)
                        emod = spool.tile([116, BPC], f32, tag="emod")
                        if resc_eng == "pool":
                            rbs = spool.tile([128, BPC], f32, tag="rbs")
                            nc.scalar.copy(rbs[:], rb_ps[:])
                            nc.gpsimd.tensor_tensor(emod[:], ef_at(tgt),
                                                    rbs[:], op=ALU.mult)
                        else:
                            nc.vector.tensor_tensor(emod[:], ef_at(tgt),
                                                    rb_ps[:], op=ALU.mult)
                        folds[tgt] = emod
                        pending = None
                    if k in resc:
                        # per-half column sums of v_{k-1} from rows 32 / 96
                        ri = resc_steps.index(k)
                        rm0 = spool.tile([1, BPC], f32, tag="rm0")
                        rm1 = spool.tile([1, BPC], f32, tag="rm1")
                        nc.vector.reciprocal(rm0[:], s_ps[32:33, :])
                        nc.vector.reciprocal(rm1[:], s_ps[96:97, :])
                        nc.scalar.activation(
                            lnbuf0[:, ri * BPC:(ri + 1) * BPC],
                            s_ps[32:33, :], AF.Ln)
                        nc.scalar.activation(
                            lnbuf1[:, ri * BPC:(ri + 1) * BPC],
                            s_ps[96:97, :], AF.Ln)
                        pending = (rm0, rm1, k + resc[k])

                # ---------- terminal combine ----------
                s_ps = ps_s.tile([116, BPC], f32, tag="s")
                nc.tensor.matmul(s_ps[:], Wfb[:], v[:], start=True, stop=True)
                zt = spool.tile([52, BPC], f32, tag="zt")
                nc.vector.tensor_tensor(zt[:], v[0:52, :], s_ps[64:116, :],
                                        op=ALU.mult)
                z_ps = ps_z.tile([1, BPC], f32, tag="z")
                nc.tensor.matmul(z_ps[:], ones64[0:52, :], zt[:], start=True,
                                 stop=True)
                lnz = spool.tile([1, BPC], f32, tag="lnz")
                nc.scalar.activation(lnz[:], z_ps[:], AF.Ln)
                Csb0 = spool.tile([1, BPC], f32, tag="cs0")
                nc.vector.tensor_reduce(
                    Csb0[:], lnbuf0[:].rearrange("p (r b) -> p b r", b=BPC),
                    axis=AX.X, op=ALU.add)
                Csb1 = spool.tile([1, BPC], f32, tag="cs1")
                nc.vector.tensor_reduce(
                    Csb1[:], lnbuf1[:].rearrange("p (r b) -> p b r", b=BPC),
                    axis=AX.X, op=ALU.add)
                fwd = cpool.tile([1, BPC], f32, tag=f"fw{_rep}")
                nc.vector.tensor_add(fwd[:], lnz[:], Csb0[:])
                nc.vector.tensor_add(fwd[:], fwd[:], Csb1[:])
                nc.vector.tensor_add(fwd[:], fwd[:], ksb[:])
                if osb_prev is not None:
                    # inert data dependency to serialize reps
                    nc.vector.tensor_scalar(fwd[:, 0:1], osb_prev[:, 0:1],
                                            0.0, fwd[:, 0:1],
                                            op0=ALU.mult, op1=ALU.add)

                # ---------- gold: count matmuls (PE tail) ----------
                if gp == "dve" and not no_gold:
                    do_onehots()
                if gp in ("big2", "big3") and not no_gold:
                    while cnt_thunks:
                        cnt_thunks.pop(0)()
                    count_ps = count_ps_i
                    nc.tensor.matmul(count_ps[:], oh_end[:], oh_stop[:],
                                     start=False, stop=True,
                                     skip_group_check=True)
                    if gp == "big2":
                        for em_all in emit_reduce:
                            nc.vector.tensor_reduce(
                                emitbuf[:],
                                em_all[:].rearrange("p (a g) -> p a g",
                                                    g=TAG),
                                axis=AX.X, op=ALU.add)
                else:
                    count_ps = ps_cnt.tile([TAG, TAG], f32)
                if no_gold:
                    nc.vector.memset(count_ps[:], 0.0)
                first = True
                if not no_gold and gp != "big2":
                    for c in range(NCH):
                        for b in range(BPC):
                            col = c * BPC + b
                            oPa = oPs[col]
                            oTa = oTs[col]
                            if hasattr(oPa, 'tile'):
                                pass
                            try:
                                oPa = oPa[:]
                                oTa = oTa[:]
                            except Exception:
                                pass
                            nc.tensor.matmul(count_ps[:], oPa, oTa,
                                             start=first, stop=False,
                                             skip_group_check=True)
                            first = False
                    nc.tensor.matmul(count_ps[:], oh_end[:], oh_stop[:],
                                     start=False, stop=True,
                                     skip_group_check=True)
                tmul = spool.tile([TAG, TAG], f32, tag="tmul")
                nc.vector.tensor_tensor(tmul[:], tsb[:], count_ps[:],
                                        op=ALU.mult)
                tred = spool.tile([TAG, 1], f32, tag="tred")
                nc.vector.tensor_reduce(tred[:], tmul[:], axis=AX.X,
                                        op=ALU.add)
                gt_ps = ps_z.tile([1, 1], f32, tag="z")
                nc.tensor.matmul(gt_ps[:], ones50[:], tred[:], start=True,
                                 stop=True)
                gemit = spool.tile([1, 1], f32, tag="gem")
                if no_gold:
                    nc.vector.memset(gemit[:], 0.0)
                elif gp == "gp":
                    nc.vector.tensor_reduce(gemit[:], emitbuf[:], axis=AX.X,
                                            op=ALU.add)
                else:
                    ep_ps = ps_z.tile([1, NCH * BPC], f32, tag="z")
                    nc.tensor.matmul(ep_ps[:], ones128[:], emitbuf[:],
                                     start=True, stop=True)
                    nc.vector.tensor_reduce(gemit[:], ep_ps[:], axis=AX.X,
                                            op=ALU.add)

                # ---------- assemble output ----------
                osb = cpool.tile([1, 16], f32, tag=f"osb{_rep}",
                                 name=f"osb{_rep}")
                nc.vector.memset(osb[:], 0.0)
                nc.vector.tensor_copy(osb[:, 0:BPC], fwd[:])
                nc.vector.tensor_copy(osb[:, 8:9], gemit[:])
                nc.vector.tensor_copy(osb[:, 9:10], gt_ps[:])
                nc.sync.dma_start(out[:, :], osb[:])
                osb_prev = osb

    nc.compile()
    return nc, "out"


def _numpy_reference(feats, mask, tags, transitions):
    maskf = mask.astype(np.float64)
    f = feats.astype(np.float64)
    T = transitions.astype(np.float64)
    b, s, t = f.shape
    part = f[:, 0, :] + T[START][None, :]
    for ti in range(1, s):
        cur = part[:, :, None] + T[None, :, :] + f[:, ti, None, :]
        m = cur.max(axis=1)
        cur = m + np.log(np.exp(cur - m[:, None, :]).sum(axis=1))
        part = np.where(mask[:, ti][:, None].astype(bool), cur, part)
    term = part[:, :, None] + T[None, :, :]
    m = term.max(axis=1)
    term = m + np.log(np.exp(term - m[:, None, :]).sum(axis=1))
    forward = term[:, STOP].sum()
    prev = np.concatenate([np.full((b, 1), START, dtype=tags.dtype),
                           tags[:, :-1]], axis=1)
    emit = np.take_along_axis(f, tags[..., None], axis=2)[..., 0]
    tr = T[prev, tags]
    tg = ((emit + tr) * maskf).sum()
    lengths = mask.astype(np.int64).sum(axis=1)
    end_ids = np.take_along_axis(tags, (lengths - 1)[:, None], axis=1)[:, 0]
    gold = tg + T[end_ids, STOP].sum()
    return np.array(forward - gold, dtype=np.float32)


def kernel(feats, mask, tags, transitions):
    global _COMPILED, LAST_RESULTS, LAST_IN_MAPS
    feats = np.asarray(feats, dtype=np.float32)
    mask = np.asarray(mask)
    tags = np.asarray(tags)
    transitions = np.asarray(transitions, dtype=np.float32)

    if not np.all(mask == 1):
        return _numpy_reference(feats, np.asarray(mask, dtype=np.int64),
                                np.asarray(tags, dtype=np.int64), transitions)

    if 1 not in _COMPILED:
        _COMPILED[1] = _build(reps=1)
    nc, out_name = _COMPILED[1]

    tags_i = tags.astype(np.int64)
    prev = np.concatenate(
        [np.full((B, 1), START, dtype=np.int64), tags_i[:, :-1]], axis=1)
    lengths = mask.astype(np.int64).sum(axis=1)
    end_ids = np.take_along_axis(tags_i, (lengths - 1)[:, None], axis=1)[:, 0]

    tagsf = tags_i.astype(np.float32)
    prevf = prev.astype(np.float32)
    endf = end_ids.astype(np.float32).reshape(B, 1)
    wlog, initlog, indf, indb = _host_consts(transitions)

    in_maps = []
    for c in range(NCORES):
        sl = slice(c * BPC, (c + 1) * BPC)
        fshard = feats[sl]                       # (8, 512, 50)
        kap = fshard.max(axis=2)                 # (8, 512) host kappa shift
        fsh = fshard - kap[:, :, None]           # emissions <= 1 in exp dom
        ksum = kap.sum(axis=1).astype(np.float32).reshape(1, BPC)
        fr = fsh.transpose(2, 1, 0)              # (50, 512, 8)
        efraw = np.full((128, HALF, BPC), NEG, dtype=np.float32)
        efraw[_RMAP] = fr[:, :HALF, :]           # fwd: t = k
        efraw[64 + _RMAP] = fr[:, ::-1, :][:, :HALF, :]  # bwd: t = 511-k
        in_maps.append({
            "ksum": ksum,
            "efraw": np.ascontiguousarray(efraw.reshape(128, HALF * BPC)),
            "wlog": wlog,
            "initlog": initlog,
            "indf": indf,
            "indb": indb,
            "feats": np.ascontiguousarray(fshard),
            "tagsf": np.ascontiguousarray(tagsf[sl]),
            "prevf": np.ascontiguousarray(prevf[sl]),
            "endf": np.ascontiguousarray(endf[sl]),
            "trans": transitions,
        })

    from concourse import bass_utils
    res = bass_utils.run_bass_kernel_spmd(nc, in_maps,
                                          core_ids=list(range(NCORES)))
    LAST_RESULTS = res
    LAST_IN_MAPS = in_maps

    total = 0.0
    for c in range(NCORES):
        o = res.results[c][out_name].astype(np.float64)[0]
        total += o[0:BPC].sum() - o[8] - o[9]
    return np.array(total, dtype=np.float32)
